# revision 40
# baseline (speedup 1.0000x reference)
"""Trainium2 Bass kernel for nn_Agent_57732950393167 (ragged_sequence).

Strategy (v2: fp16 data path)
-----------------------------
Data-parallel over batches: 32 batches / 8 cores = 4 batches ("groups" g)
per core, each with V=8 vehicles -> 32 vehicles/core.

The v1 kernel was PE-bound: fp32 moving operands stream at 2 cycles per
element on the PE, fp32 transposes and fp32-stationary matmuls run as
double (LOW+HIGH) passes, and LDWEIGHTS of fp32 stationaries cannot use
fast-weight-load.  v2 moves the whole heavy data path to fp16:

 * All large inputs ship as fp16 (halves HBM traffic to ~4.8 MB/core)
   and all large matmuls run with fp16 stationary+moving operands
   (1 cycle/element, 4x fast-weight-load for 128-col stationaries,
   single-pass transposes).  PSUM accumulation stays fp32.
 * Numerically validated offline: with fp16 rounding applied to every
   input AND every on-device cast point (query, qw, attention weights,
   AF, heads, final_Q, fw) the flat-softmax argmax of all 32 batches is
   unchanged and the min top-2 gap stays 6.5e-4 (fp64 ref: 4.1e-4).
   bf16 flips one batch -- fp16 is the floor.
 * nde = ndf @ W_ns ([T,N,384]) is never materialized (rank-8 folding
   into compat / heads / logits, as in v1).
 * Single sync-HWDGE DMA ring in consumption order; transfers merged
   into 11 issues (consts+weights fp16/f32, kt+ndftm group-pair halves,
   rhsha per pair, lt halves, mask).
 * Phase-A small ops moved from ACT to DVE (broadcast copies / STT) so
   the ACT queue reaches the first softmax exp immediately after C0.
 * Softmax runs unnormalized; 1/sum folded into the heads PSUM rescale.
 * log(mask) approximated by MASK_BIG*(mask-1), MASK_BIG=50 (fp16-exact).
"""

import numpy as np

B, N, D, H, V = 32, 1024, 128, 8, 8
KS = D // H            # 16
F_V = 4
F_ND = 8
TANH_CLIP = 10.0
MASK_BIG = 50.0
NCORES = 8
G = B // NCORES        # 4 groups (batches) per core
NPAIR = G // 2         # 2 batch-pairs per core

_PROGRAM_CACHE = {}

# fp16 const+weight pack A: everything phase A / C needs (cols)
CP16_REPL = 0          # [8,128]  eye(8) tiled 16x horizontally
CP16_HSELB = 128       # [128,128]
CP16_REPLBIG = 256     # [8,64]
CP16_WCSHI = 320       # [128,128]
CP16_WCSLO = 448       # [4,128]
CP16_VDFT = 576        # [4,32]
CP16_WNSKT = 608       # [128,8]
CP16A_W = 616

# fp16 const+weight pack B: late-use (T/sm phases)
CP16_IDENT = 0         # [128,128] identity
CP16_WOUT = 128        # [128,128]
CP16_WNSV = 256        # [8,128]
CP16_WNSLT = 384       # [128,8]
CP16B_W = 392

# f32 const pack A: phase A needs (cols)
CPF_FCT = 0            # [128,4]
CPF_BDSEL = 4          # [128,128]
CPFA_W = 132

# f32 const pack B: late-use
CPF_DIAG = 0           # [128,128]
CPF_IDENTPAD = 128     # [128,64]
CPF_HSEL = 192         # [128,64]
CPF_FWSEL = 256        # [128,16]
CPF_IDENT32 = 272      # [32,32]
CPF_C8K = 304          # [32,1]
CPFB_W = 305


def _build_cpack16a():
    cp = np.zeros((128, CP16A_W), dtype=np.float16)
    cp[0:F_ND, CP16_REPL:CP16_REPL + 128] = np.tile(
        np.eye(F_ND, dtype=np.float16), (1, 16))
    hb = np.zeros((128, 128), dtype=np.float16)
    for d in range(128):
        h = d // KS
        for g2 in range(2):
            hb[d, g2 * 64 + h * V:g2 * 64 + (h + 1) * V] = 1.0
    cp[:, CP16_HSELB:CP16_HSELB + 128] = hb
    rb = np.zeros((V, H * V), dtype=np.float16)
    for v in range(V):
        rb[v, v::V] = 1.0
    cp[0:V, CP16_REPLBIG:CP16_REPLBIG + 64] = rb
    return cp


def _build_cpack16b():
    cp = np.zeros((128, CP16B_W), dtype=np.float16)
    cp[:, CP16_IDENT:CP16_IDENT + 128] = np.eye(128, dtype=np.float16)
    return cp


def _build_cpackfa():
    return np.zeros((128, CPFA_W), dtype=np.float32)


def _build_cpackfb():
    cp = np.zeros((128, CPFB_W), dtype=np.float32)
    dm = np.zeros((128, 128), dtype=np.float32)
    for p in range(128):
        g2, hv = divmod(p, 64)
        a = g2 * 8 + (hv % 8)
        dm[p, a * 8:(a + 1) * 8] = 1.0
    cp[:, CPF_DIAG:CPF_DIAG + 128] = dm
    cp[:, CPF_IDENTPAD:CPF_IDENTPAD + 64] = np.tile(
        np.eye(64, dtype=np.float32), (2, 1))
    hs = np.zeros((128, 64), dtype=np.float32)
    for hk in range(128):
        h = hk // KS
        hs[hk, h * 8:(h + 1) * 8] = 1.0
    cp[:, CPF_HSEL:CPF_HSEL + 64] = hs
    fw = np.zeros((128, 16), dtype=np.float32)
    for p in range(128):
        fw[p, p // 8] = 1.0
    cp[:, CPF_FWSEL:CPF_FWSEL + 16] = fw
    cp[0:32, CPF_IDENT32:CPF_IDENT32 + 32] = np.eye(32, dtype=np.float32)
    cp[0:32, CPF_C8K] = 8192.0 - 1024.0 * (np.arange(32) % 8)
    return cp


def _build_bdsel():
    bd = np.zeros((128, 128), dtype=np.float32)
    for p in range(128):
        a = p // 8
        g2, v = divmod(a, 8)
        bd[p, g2 * 64 + v:g2 * 64 + 64:8] = 1.0
    return bd


# --------------------------------------------------------------------------
# Device program
# --------------------------------------------------------------------------

def _build_program():
    import contextlib

    import concourse.bacc as bacc
    import concourse.tile as tile
    import concourse.mybir as mybir

    dt = mybir.dt
    f32 = dt.float32
    f16 = dt.float16
    AF_EXP = mybir.ActivationFunctionType.Exp
    AF_LN = mybir.ActivationFunctionType.Ln
    AF_TANH = mybir.ActivationFunctionType.Tanh
    AF_COPY = mybir.ActivationFunctionType.Copy
    OP = mybir.AluOpType
    AX = mybir.AxisListType

    nc = bacc.Bacc("TRN2", target_bir_lowering=False, debug=False,
                   num_devices=NCORES)

    # ---- external inputs (per-core shards, host-prepped layouts) ----
    cp16a_in = nc.dram_tensor("cp16a_in", [128, CP16A_W], f16,
                              kind="ExternalInput")
    cp16b_in = nc.dram_tensor("cp16b_in", [128, CP16B_W], f16,
                              kind="ExternalInput")
    cpfa_in = nc.dram_tensor("cpfa_in", [128, CPFA_W], f32,
                             kind="ExternalInput")
    cpfb_in = nc.dram_tensor("cpfb_in", [128, CPFB_W], f32,
                             kind="ExternalInput")
    kt_in = nc.dram_tensor("kt_in", [128, G * N], f16, kind="ExternalInput")
    nd_in = nc.dram_tensor("nd_in", [72, G * N], f16, kind="ExternalInput")
    rh_in = nc.dram_tensor("rh_in", [128, NPAIR * 3 * N], f16,
                           kind="ExternalInput")
    lt_in = nc.dram_tensor("lt_in", [128, G * N], f16, kind="ExternalInput")
    mbs32_in = nc.dram_tensor("mbs32_in", [32, N], f32,
                              kind="ExternalInput")

    res_out = nc.dram_tensor("res_out", [G, 4], f32, kind="ExternalOutput")

    with tile.TileContext(nc) as tc:
        with contextlib.ExitStack() as ctx:
            sb = ctx.enter_context(tc.tile_pool(name="sb", bufs=1))
            scr = ctx.enter_context(tc.tile_pool(name="scr", bufs=4))
            acc = ctx.enter_context(
                tc.tile_pool(name="acc", bufs=2, space="PSUM"))
            tp = ctx.enter_context(
                tc.tile_pool(name="tp", bufs=3, space="PSUM"))
            flp = ctx.enter_context(
                tc.tile_pool(name="flp", bufs=1, space="PSUM"))

            def P(name, shape, dtype=f32):
                return sb.tile(shape, dtype, name=name, tag=name)

            def S(name, shape, dtype=f32):
                if shape[-1] >= 512:
                    return scr.tile(shape, dtype, name=name, tag="sbig",
                                    bufs=4)
                return scr.tile(shape, dtype, name=name, tag="ssml", bufs=8)

            # ================= persistent SBUF tiles =================
            cp16a = P("cp16a", [128, CP16A_W], f16)
            cp16b = P("cp16b", [128, CP16B_W], f16)
            cpfa = P("cpfa", [128, CPFA_W], f32)
            cpfb = P("cpfb", [128, CPFB_W], f32)
            kt = P("kt", [128, G * N], f16)
            nd = P("nd", [72, G * N], f16)
            rh = P("rh", [128, NPAIR * 3 * N], f16)
            lt = P("lt", [128, G * N], f16)
            mbs32 = P("mbs32", [32, N], f32)
            attnt = [P(f"attntp{p}", [128, N], f16) for p in range(NPAIR)]
            attnnt = [P(f"attnnt{p}", [128, 8 * 128], f16)
                      for p in range(NPAIR)]
            fctq = P("fctq", [128, G])
            fct8a = P("fct8a", [128, G * V], f16)
            queryt = P("queryt", [128, G * V], f16)  # 0.25-scaled query^T
            blockq = [P(f"blockq{p}", [128, 128], f16) for p in range(NPAIR)]
            bdq72 = [P(f"bdq72_{g}", [72, 64], f16) for g in range(G)]
            ha_sb = [P(f"hasb{p}", [128, 384]) for p in range(NPAIR)]
            afdt = [P(f"afdt{p}", [F_ND, 128], f16) for p in range(NPAIR)]
            hct = [P(f"hctp{p}", [128, 2 * V], f16) for p in range(NPAIR)]
            fqt = [P(f"fqt{p}", [128, 2 * V], f16) for p in range(NPAIR)]
            fq32g = [P(f"fq32g{g}", [128, 32], f16) for g in range(G)]
            bdfw32 = [P(f"bdfw32_{g}", [64, 32], f16) for g in range(G)]
            lgf32 = P("lgf32", [32, N])
            u32 = P("u32", [32, N])
            rinv_p = [P(f"rinvp{p}", [128, 1]) for p in range(NPAIR)]
            prime16 = P("prime16", [128, 256], f16)

            # ================= DMA issues =================
            # Everything rides ONE HWDGE ring (sync) in exact consumption
            # order: a single queue gets the full per-NC HBM bandwidth and
            # drains strictly FIFO, so arrival order == this issue order.
            nc.sync.dma_start(cp16a[:], cp16a_in.ap())
            nc.sync.dma_start(cpfa[:], cpfa_in.ap())
            nc.sync.dma_start(kt[:, 0:2 * N], kt_in.ap()[:, 0:2 * N])
            nc.sync.dma_start(nd[:, 0:2 * N], nd_in.ap()[:, 0:2 * N])
            nc.sync.dma_start(kt[:, 2 * N:4 * N], kt_in.ap()[:, 2 * N:4 * N])
            nc.sync.dma_start(nd[:, 2 * N:4 * N], nd_in.ap()[:, 2 * N:4 * N])
            nc.sync.dma_start(cp16b[:], cp16b_in.ap())
            nc.sync.dma_start(cpfb[:], cpfb_in.ap())
            nc.sync.dma_start(rh[:, 0:3 * N], rh_in.ap()[:, 0:3 * N])
            nc.sync.dma_start(rh[:, 3 * N:6 * N], rh_in.ap()[:, 3 * N:6 * N])
            nc.sync.dma_start(lt[:, 0:2 * N], lt_in.ap()[:, 0:2 * N])
            nc.sync.dma_start(lt[:, 2 * N:4 * N], lt_in.ap()[:, 2 * N:4 * N])
            nc.sync.dma_start(mbs32[:], mbs32_in.ap())

            # ================= small setup on DVE ==================
            res16 = P("res16", [1, 4 * G])
            nc.vector.memset(prime16[:], 0.0)
            nc.vector.memset(res16[:, 3:16:4], 0.0)
            for g in range(G):
                nc.vector.memset(fq32g[g][:], 0.0)
                nc.vector.memset(bdfw32[g][:], 0.0)

            # PE warm-up: a short back-to-back fp16 matmul chain bridges
            # until phase A's data arrives; A + C0 continue the activity
            # so the HAM SHORT window sees ~3.4us sustained and
            # un-throttles early.  No data-dependent primes here -- a
            # prime waiting on a DMA would stall the strict PE queue and
            # block phase A behind it (measured: 4us lost).
            for i in range(6):
                prime_ps = flp.tile([128, 256], f32, name=f"prime{i}",
                                    tag="fl")
                nc.tensor.matmul(prime_ps[:], prime16[:, 0:128],
                                 prime16[:], start=True, stop=True,
                                 skip_group_check=True)

            # HAM keep-warm fillers: cheap fp16 junk matmuls into a
            # dedicated PSUM bank.  The tile scheduler floats dep-free
            # instructions, so fillers must READ a recently-produced tile
            # to stay pinned at their program point.
            fl_ps = flp.tile([128, 256], f32, name="fl_ps", tag="fl")

            def filler(n, early=False):
                for _ in range(n):
                    if early:
                        nc.tensor.matmul(fl_ps[:], prime16[:, 0:128],
                                         prime16[:],
                                         start=True, stop=True,
                                         skip_group_check=True)
                    else:
                        nc.tensor.matmul(fl_ps[:], kt[:, 0:128],
                                         kt[:, 0:256],
                                         start=True, stop=True,
                                         skip_group_check=True)

            def filler_on(st, mv, m, w):
                # junk matmul reading `st` (stationary [K, m]) and `mv`
                # (moving [K, w]) so the scheduler cannot hoist it ahead of
                # the ops that produce them
                nc.tensor.matmul(fl_ps[0:m, 0:w], st, mv,
                                 start=True, stop=True,
                                 skip_group_check=True)

            # const/weight slices
            repl = cp16a[0:F_ND, CP16_REPL:CP16_REPL + 128]
            hselb = cp16a[:, CP16_HSELB:CP16_HSELB + 128]
            replbig = cp16a[0:V, CP16_REPLBIG:CP16_REPLBIG + 64]
            wcs_hi = cp16a[:, CP16_WCSHI:CP16_WCSHI + 128]
            wcs_lo = cp16a[0:F_V, CP16_WCSLO:CP16_WCSLO + 128]
            vdft = cp16a[0:F_V, CP16_VDFT:CP16_VDFT + 32]
            wnskt = cp16a[:, CP16_WNSKT:CP16_WNSKT + F_ND]
            ident16 = cp16b[:, CP16_IDENT:CP16_IDENT + 128]
            wout = cp16b[:, CP16_WOUT:CP16_WOUT + 128]
            wnsv = cp16b[0:F_ND, CP16_WNSV:CP16_WNSV + 128]
            wnslt = cp16b[:, CP16_WNSLT:CP16_WNSLT + F_ND]
            fct = cpfa[:, CPF_FCT:CPF_FCT + 4]
            bdsel = cpfa[:, CPF_BDSEL:CPF_BDSEL + 128]
            diagmask = cpfb[:, CPF_DIAG:CPF_DIAG + 128]
            identpad = cpfb[:, CPF_IDENTPAD:CPF_IDENTPAD + 64]
            hsel = cpfb[:, CPF_HSEL:CPF_HSEL + 64]
            fwsel = cpfb[:, CPF_FWSEL:CPF_FWSEL + 16]
            ident32 = cpfb[0:32, CPF_IDENT32:CPF_IDENT32 + 32]
            c8k32 = cpfb[0:32, CPF_C8K:CPF_C8K + 1]

            # fctq = 0.25*fc^T (f32, used as ACT/STT bias)
            nc.vector.tensor_scalar_mul(fctq[:], fct, 0.25)

            # ================= phase A: query / qw smalls =================
            # fct8a[:, (g,v)] = fc[:, g]  (broadcast copy on DVE)
            nc.vector.tensor_copy(
                fct8a.rearrange("d (g v) -> d g v", g=G),
                fct.unsqueeze(2).broadcast_to([128, G, V]))
            qt_ps = tp.tile([128, G * V], f32, name="qt_ps", tag="tp")
            nc.tensor.matmul(qt_ps[:], wcs_hi, fct8a[:],
                             start=True, stop=False, skip_group_check=True)
            nc.tensor.matmul(qt_ps[:], wcs_lo, vdft,
                             start=False, stop=True, skip_group_check=True)
            filler(3, early=True)
            # queryt = 0.25*(cur + fc) = 0.25*qt + fctq   (DVE STT)
            nc.vector.scalar_tensor_tensor(
                queryt.rearrange("d (g v) -> d g v", g=G),
                qt_ps.rearrange("d (g v) -> d g v", g=G), 0.25,
                fctq.unsqueeze(2).broadcast_to([128, G, V]),
                op0=OP.mult, op1=OP.add)

            for p in range(NPAIR):
                # blockq[d, (g2,h,v)] = queryt[d, (g,v)] * (h == d//16)
                qview = (queryt[:, 2 * p * V:(2 * p + 2) * V]
                         .rearrange("d (g2 v) -> d g2 v", g2=2)
                         .unsqueeze(2).broadcast_to([128, 2, H, V]))
                nc.vector.tensor_tensor(
                    blockq[p].rearrange("d (g2 h v) -> d g2 h v", g2=2, h=H),
                    qview, hselb.rearrange("d (g2 h v) -> d g2 h v",
                                           g2=2, h=H),
                    OP.mult)
                # qw_all[f, (g2,h,v)] then replicate+mask into block-diag
                qw_ps = tp.tile([F_ND, 128], f32, name=f"qw_ps{p}", tag="tp")
                nc.tensor.matmul(qw_ps[:], wnskt, blockq[p][:],
                                 start=True, stop=True)
                qw_sbt = S(f"qw_sbt{p}", [F_ND, 128], f16)
                nc.vector.tensor_copy(qw_sbt[:], qw_ps[:])
                qwr_ps = tp.tile([128, 128], f32, name=f"qwr_ps{p}", tag="tp")
                nc.tensor.matmul(qwr_ps[:], repl, qw_sbt[:],
                                 start=True, stop=True)
                filler(2, early=True)
                for g2 in range(2):
                    g = 2 * p + g2
                    gsl = slice(g2 * 64, (g2 + 1) * 64)
                    nc.vector.tensor_tensor(bdq72[g][0:64, :],
                                            qwr_ps[gsl, gsl],
                                            bdsel[gsl, gsl], OP.mult)
                    nc.vector.tensor_copy(bdq72[g][64:72, :], replbig)

            # ===== phases C/T/H/sm =====
            def phase_C(g):
                p, g2 = divmod(g, 2)
                gsl = slice(g2 * 64, (g2 + 1) * 64)
                compat = acc.tile([64, N], f32, name=f"compat{g}",
                                  tag="acc")
                # static first (kt arrives before nd in the stream);
                # dyn + mask in one 72-row contraction after
                for half in range(2):
                    sl = slice(g * N + half * 512, g * N + (half + 1) * 512)
                    osl = slice(half * 512, (half + 1) * 512)
                    nc.tensor.matmul(
                        compat[:, osl], blockq[p][:, gsl], kt[:, sl],
                        start=True, stop=False, skip_group_check=True)
                for half in range(2):
                    sl = slice(g * N + half * 512, g * N + (half + 1) * 512)
                    osl = slice(half * 512, (half + 1) * 512)
                    nc.tensor.matmul(
                        compat[:, osl], bdq72[g][:], nd[:, sl],
                        start=False, stop=True, skip_group_check=True)
                # unnormalized exp into the pair tile (|compat| < ~15)
                rsum = S(f"rsum{g}", [64, 1])
                nc.scalar.activation(attnt[p][gsl, :], compat[:],
                                     AF_EXP, accum_out=rsum[:])
                nc.vector.reciprocal(rinv_p[p][gsl, :], rsum[:])

            def phase_T(p):
                # attn^T -> attn_n: fp16 [128,128] transposes, paired copies
                for c2 in range(4):
                    at_ps = tp.tile([128, 256], f16,
                                    name=f"at_ps{p}_{c2}", tag="tp")
                    for j in range(2):
                        c = 2 * c2 + j
                        nc.tensor.matmul(
                            at_ps[:, j * 128:(j + 1) * 128],
                            attnt[p][:, c * 128:(c + 1) * 128],
                            ident16,
                            is_transpose=True,
                            start=True, stop=True,
                            skip_group_check=True)
                    dst = (attnnt[p]
                           .rearrange("q (c w) -> q c w", w=128)
                           [:, 2 * c2:2 * c2 + 2, :])
                    src_ap = at_ps.rearrange("q (j w) -> q j w", j=2)
                    if c2 % 2 == 0:
                        nc.scalar.activation(dst, src_ap, AF_COPY)
                    else:
                        nc.vector.tensor_copy(dst, src_ap)
                filler_on(attnnt[p][:, 0:16], attnnt[p][:, 0:256], 16, 256)

            def phase_H(p):
                # heads+AF over the 3 contiguous rh regions (2-dim free
                # AP); normalization via rinv in the PSUM->SBUF move
                ha_ps = tp.tile([128, 384], f32, name=f"ha_ps{p}", tag="tp")
                rhp = (rh[:, p * 3 * N:(p + 1) * 3 * N]
                       .rearrange("q (r w) -> q r w", r=3))
                for c in range(8):
                    nc.tensor.matmul(ha_ps[:],
                                     attnnt[p][:, c * 128:(c + 1) * 128],
                                     rhp[:, :, c * 128:(c + 1) * 128],
                                     start=(c == 0), stop=(c == 7))
                nc.vector.tensor_scalar_mul(ha_sb[p][:], ha_ps[:],
                                            rinv_p[p][:])
                filler_on(ha_sb[p][:, 0:16], ha_sb[p][:, 0:256], 16, 256)

            def phase_sm(p):
                # AF diag-extract -> AFd [128, F] -> AFd^T (fp16)
                aftmp = S(f"aftmp{p}", [128, 128])
                nc.vector.tensor_tensor(aftmp[:], ha_sb[p][:, 256:384],
                                        diagmask, OP.mult)
                afd = S(f"afd{p}", [128, F_ND], f16)
                with nc.allow_low_precision("fp16 AF validated offline"):
                    nc.vector.tensor_reduce(
                        afd[:], aftmp.rearrange("q (a f) -> q f a", f=F_ND),
                        AX.X, OP.add)
                filler_on(afd[:, 0:8], attnnt[p][:, 0:256], 8, 256)
                afd_ps = tp.tile([F_ND, 128], f16, name=f"afd_ps{p}",
                                 tag="tp")
                nc.tensor.matmul(afd_ps[:], afd[:], ident16,
                                 is_transpose=True, start=True, stop=True)
                nc.vector.tensor_copy(afdt[p][:], afd_ps[:])

                # heads -> hcT -> final_Q^T per group
                fqp = tp.tile([128, 2 * V], f32, name=f"fqp{p}", tag="tp")
                for g2 in range(2):
                    g = 2 * p + g2
                    hq_ps = tp.tile([128, 64], f32, name=f"hq_ps{g}",
                                    tag="tp")
                    nc.tensor.matmul(
                        hq_ps[:],
                        ha_sb[p][g2 * 64:(g2 + 1) * 64,
                                 g2 * 128:(g2 + 1) * 128],
                        identpad[g2 * 64:(g2 + 1) * 64, :],
                        is_transpose=True, start=True, stop=False,
                        skip_group_check=True)
                    nc.tensor.matmul(
                        hq_ps[:], wnsv,
                        afdt[p][:, g2 * 64:(g2 + 1) * 64],
                        start=False, stop=True, skip_group_check=True)
                    hqs = S(f"hqs{g}", [128, 64])
                    nc.vector.tensor_tensor(hqs[:], hq_ps[:], hsel,
                                            OP.mult)
                    with nc.allow_low_precision("fp16 heads validated offline"):
                        nc.vector.tensor_reduce(
                            hct[p][:, g2 * V:(g2 + 1) * V],
                            hqs.rearrange("q (hh v) -> q v hh", v=V),
                            AX.X, OP.add)
                    filler_on(hct[p][:, g2 * V:g2 * V + V],
                              attnnt[p][:, 0:256], V, 256)
                nc.tensor.matmul(fqp[:], wout, hct[p][:],
                                 start=True, stop=True)
                nc.vector.tensor_copy(fqt[p][:], fqp[:])
                for g2 in range(2):
                    g = 2 * p + g2
                    nc.vector.tensor_copy(
                        fq32g[g][:, g * 8:(g + 1) * 8],
                        fqp[:, g2 * V:(g2 + 1) * V])

                # block-diag fw
                fw_ps = tp.tile([F_ND, 2 * V], f32, name=f"fw_ps{p}",
                                tag="tp")
                nc.tensor.matmul(fw_ps[:], wnslt, fqt[p][:],
                                 start=True, stop=True)
                fw_sbt = S(f"fw_sbt{p}", [F_ND, 2 * V], f16)
                nc.vector.tensor_copy(fw_sbt[:], fw_ps[:])
                filler_on(fqt[p][:, 0:16], attnnt[p][:, 0:256], 16, 256)
                fwr_ps = tp.tile([128, 2 * V], f32, name=f"fwr_ps{p}",
                                 tag="tp")
                nc.tensor.matmul(fwr_ps[:], repl, fw_sbt[:],
                                 start=True, stop=True)
                for g2 in range(2):
                    g = 2 * p + g2
                    gsl = slice(g2 * 64, (g2 + 1) * 64)
                    vsl = slice(g2 * V, (g2 + 1) * V)
                    nc.vector.tensor_tensor(
                        bdfw32[g][:, g * 8:(g + 1) * 8],
                        fwr_ps[gsl, vsl], fwsel[gsl, vsl], OP.mult)

            # ======== phase E: batched logits for all 4 groups ===========
            # lg32[8g+v, n] = logits of (group g, vehicle v): group g's
            # stationaries are zero-padded to 32 cols at offset 8g so all
            # 16 matmuls accumulate into one [32, N] PSUM tile.  Emitted in
            # two clumps (pair 0's groups after sm0, pair 1's after sm1) so
            # the PE streams pair 0's logits while sm1's DVE chain runs.
            # Stationary-major: each stationary loads once and streams both
            # 512-halves back-to-back.  (lg32 is allocated after the compat
            # tiles so the acc pool's slot rotation stays forward-ordered.)
            def phase_E(p, lg32):
                for g in (2 * p, 2 * p + 1):
                    for half in range(2):
                        osl = slice(half * 512, (half + 1) * 512)
                        sl = slice(g * N + half * 512,
                                   g * N + (half + 1) * 512)
                        nc.tensor.matmul(
                            lg32[:, osl], bdfw32[g][:], nd[0:64, sl],
                            start=(g == 0), stop=False,
                            skip_group_check=True)
                for g in (2 * p, 2 * p + 1):
                    for half in range(2):
                        osl = slice(half * 512, (half + 1) * 512)
                        sl = slice(g * N + half * 512,
                                   g * N + (half + 1) * 512)
                        nc.tensor.matmul(
                            lg32[:, osl], fq32g[g][:], lt[:, sl],
                            start=False,
                            stop=(p == 1 and g == G - 1 and half == 1),
                            skip_group_check=True)

            phase_C(0)
            phase_C(1)
            # arrival-pinned fillers bridge the PE while C2 waits for its
            # WAR slot (exp0); they gate on the same DMA data C2 needs
            filler_on(kt[0:72, 2 * N:2 * N + 16],
                      kt[0:72, 2 * N:2 * N + 256], 16, 256)
            phase_C(2)
            filler_on(nd[0:72, 3 * N:3 * N + 16],
                      nd[0:72, 3 * N:3 * N + 256], 16, 256)
            phase_C(3)
            phase_T(0)
            phase_T(1)
            phase_H(0)
            phase_sm(0)
            lg32 = acc.tile([32, N], f32, name="lg32", tag="acc")
            phase_H(1)
            phase_E(0, lg32)
            phase_sm(1)
            phase_E(1, lg32)

            # lgf = 10*tanh(x/sqrt(D)) + mask, in halves so the DVE
            # mask-add of half 0 overlaps the ACT tanh of half 1
            for half in range(2):
                sl = slice(half * 512, (half + 1) * 512)
                nc.scalar.activation(u32[:, sl], lg32[:, sl], AF_TANH,
                                     scale=float(1.0 / np.sqrt(D)))
                nc.vector.scalar_tensor_tensor(
                    lgf32[:, sl], u32[:, sl], TANH_CLIP, mbs32[:, sl],
                    op0=OP.mult, op1=OP.add)

            # ============ epilogue: batched flat log-softmax/argmax ======
            # cde[32, 4] packs (max, rowsum, candidate) so a single PE
            # transpose brings all three into free-dim layout at once.
            cde = P("cde", [32, 4])
            rs32h = S("rs32h", [32, 2])
            expf = S("expfe", [32, N])
            for half in range(2):
                sl = slice(half * 512, (half + 1) * 512)
                nc.scalar.activation(expf[:, sl], lgf32[:, sl], AF_EXP,
                                     accum_out=rs32h[:, half:half + 1])
            # prefetch the Ln/Exp ACT table while the DVE argmax chain runs
            lndmy = S("lndmy", [1, 1])
            nc.scalar.activation(lndmy[:], rs32h[0:1, 0:1], AF_LN)
            nc.vector.tensor_tensor(cde[:, 1:2], rs32h[:, 0:1],
                                    rs32h[:, 1:2], OP.add)
            mx8 = S("mx8e", [32, 8])
            ix8 = S("ix8e", [32, 8], dt.uint32)
            nc.vector.max_with_indices(mx8[:], ix8[:], lgf32[:])
            nc.vector.tensor_copy(cde[:, 0:1], mx8[:, 0:1])
            idxf = S("idxfe", [32, 1])
            nc.vector.tensor_copy(idxf[:], ix8[:, 0:1])
            nc.vector.tensor_tensor(cde[:, 2:3], c8k32, idxf[:],
                                    OP.subtract)

            cde_ps = tp.tile([1, 96], f32, name="cde_ps", tag="tp")
            for j in range(3):
                nc.tensor.matmul(cde_ps[:, j * 32:(j + 1) * 32],
                                 cde[:, j:j + 1], ident32,
                                 is_transpose=True, start=True, stop=True,
                                 skip_group_check=True)
            cdet = S("cdete", [1, 96])
            nc.vector.tensor_copy(cdet[:], cde_ps[:])
            rmt = cdet[0:1, 0:32]
            rst = cdet[0:1, 32:64]
            cdt = cdet[0:1, 64:96]

            mt4 = S("mt4e", [1, G])
            nc.vector.tensor_reduce(mt4[:],
                                    rmt.rearrange("o (g v) -> o g v", g=G),
                                    AX.X, OP.max)
            s4 = S("s4e", [1, G])
            nc.vector.tensor_reduce(s4[:],
                                    rst.rearrange("o (g v) -> o g v", g=G),
                                    AX.X, OP.add)
            # lp = M - ln(S) directly (S = sum of unshifted exps, < 4e7,
            # safely inside f32/ACT-Ln range); prob = exp(lp)
            lns4 = S("lns4e", [1, G])
            nc.scalar.activation(lns4[:], s4[:], AF_LN)
            # cand chain on DVE overlaps the Ln ACT hop
            mtb = (mt4.unsqueeze(2).broadcast_to([1, G, V]))
            eq = S("eqe", [1, 32])
            nc.vector.tensor_tensor(
                eq.rearrange("o (g v) -> o g v", g=G),
                rmt.rearrange("o (g v) -> o g v", g=G), mtb, OP.is_equal)
            cs = S("cse", [1, 32])
            nc.vector.tensor_tensor(cs[:], eq[:], cdt, OP.mult)
            cm4 = S("cm4e", [1, G])
            nc.vector.tensor_reduce(cm4[:],
                                    cs.rearrange("o (g v) -> o g v", g=G),
                                    AX.X, OP.max)
            nc.vector.tensor_scalar(res16[:, 0:16:4], cm4[:], -1.0, 8192.0,
                                    OP.mult, OP.add)
            nc.vector.tensor_tensor(res16[:, 1:16:4], mt4[:], lns4[:],
                                    OP.subtract)
            prob4 = S("prob4e", [1, G])
            nc.scalar.activation(prob4[:], res16[:, 1:16:4], AF_EXP)
            nc.vector.scalar_tensor_tensor(
                res16[:, 2:16:4], prob4[:], -1.0, res16[:, 1:16:4],
                op0=OP.mult, op1=OP.mult)
            nc.sync.dma_start(
                res_out.ap().rearrange("a b -> (a b)").unsqueeze(0),
                res16[:])

    nc.compile()
    return nc


def _get_program():
    if "nc" not in _PROGRAM_CACHE:
        _PROGRAM_CACHE["nc"] = _build_program()
    return _PROGRAM_CACHE["nc"]


# --------------------------------------------------------------------------
# Host-side sharding / layout prep
# --------------------------------------------------------------------------

def _make_in_maps(inputs):
    gk = np.asarray(inputs["glimpse_K_static"], dtype=np.float32)
    gv = np.asarray(inputs["glimpse_V_static"], dtype=np.float32)
    lk = np.asarray(inputs["logit_K_static"], dtype=np.float32)
    ndf = np.asarray(inputs["node_dynamic_features"], dtype=np.float32)
    vdf = np.asarray(inputs["vehicle_dynamic_features"], dtype=np.float32)
    fc = np.asarray(inputs["fixed_context"], dtype=np.float32)
    msk = np.asarray(inputs["feasibility_mask"])
    w_cs = np.asarray(inputs["W_cs"], dtype=np.float32)
    w_ns = np.asarray(inputs["W_ns"], dtype=np.float32)
    w_out = np.asarray(inputs["W_out"], dtype=np.float32)

    cp16a_base = _build_cpack16a()
    cp16b_base = _build_cpack16b()
    cpfa_base = _build_cpackfa()
    cpfa_base[:, CPF_BDSEL:CPF_BDSEL + 128] = _build_bdsel()
    cpfb_base = _build_cpackfb()

    in_maps = []
    for c in range(NCORES):
        bs = slice(c * G, (c + 1) * G)
        kt = np.ascontiguousarray(
            gk[:, bs].transpose(1, 0, 3, 2).reshape(G, 128, N)
            .transpose(1, 0, 2).reshape(128, G * N)).astype(np.float16)
        lt = np.ascontiguousarray(
            lk[bs].transpose(0, 2, 1).transpose(1, 0, 2)
            .reshape(128, G * N)).astype(np.float16)
        vn = gv[:, bs].transpose(1, 2, 0, 3).reshape(G, N, 128)
        ndd = ndf[bs]                                   # [G, V, N, F]
        ndtm = np.zeros((G, 72, N), dtype=np.float16)
        ndtm[:, 0:64, :] = ndd.transpose(0, 1, 3, 2).reshape(G, 64, N)
        mbx = (msk[bs].astype(np.float32) - 1.0) * MASK_BIG   # [G, V, N]
        ndtm[:, 64:72, :] = mbx
        nd_in = np.ascontiguousarray(
            ndtm.transpose(1, 0, 2).reshape(72, G * N))
        mbs32 = np.ascontiguousarray(mbx.reshape(32, N)).astype(np.float32)
        ndfn = (ndd.reshape(NPAIR, 2, V, N, F_ND)
                .transpose(0, 3, 1, 2, 4).reshape(NPAIR, N, 128))
        rhsha = np.empty((NPAIR, 128, 24, 128), dtype=np.float16)
        for p in range(NPAIR):
            for g2 in range(2):
                rhsha[p, :, g2 * 8:(g2 + 1) * 8, :] = (
                    vn[2 * p + g2].reshape(8, 128, 128).transpose(1, 0, 2))
            rhsha[p, :, 16:24, :] = (
                ndfn[p].reshape(8, 128, 128).transpose(1, 0, 2))
        rh_in = np.ascontiguousarray(
            rhsha.reshape(NPAIR, 128, 3 * N)
            .transpose(1, 0, 2).reshape(128, NPAIR * 3 * N))

        cp16a = cp16a_base.copy()
        cp16a[:, CP16_WCSHI:CP16_WCSHI + 128] = w_cs[:D].astype(np.float16)
        cp16a[0:F_V, CP16_WCSLO:CP16_WCSLO + 128] = (
            w_cs[D:].astype(np.float16))
        cp16a[0:F_V, CP16_VDFT:CP16_VDFT + 32] = (
            vdf[bs].transpose(2, 0, 1).reshape(F_V, 32).astype(np.float16))
        cp16a[:, CP16_WNSKT:CP16_WNSKT + F_ND] = (
            w_ns[:, D:2 * D].T.astype(np.float16))
        cp16b = cp16b_base.copy()
        cp16b[:, CP16_WOUT:CP16_WOUT + 128] = w_out.astype(np.float16)
        cp16b[0:F_ND, CP16_WNSV:CP16_WNSV + 128] = (
            w_ns[:, 0:D].astype(np.float16))
        cp16b[:, CP16_WNSLT:CP16_WNSLT + F_ND] = (
            w_ns[:, 2 * D:3 * D].T.astype(np.float16))
        cpfa = cpfa_base.copy()
        cpfa[:, CPF_FCT:CPF_FCT + 4] = fc[bs].T

        in_maps.append({
            "cp16a_in": cp16a,
            "cp16b_in": cp16b,
            "cpfa_in": cpfa,
            "cpfb_in": cpfb_base,
            "kt_in": kt,
            "nd_in": nd_in,
            "rh_in": rh_in,
            "lt_in": lt,
            "mbs32_in": mbs32,
        })
    return in_maps


def _postprocess(res_list):
    out = np.concatenate(res_list, axis=0)              # [B, 4]
    a = out[:, 0]
    lp = out[:, 1].astype(np.float32)
    ent = out[:, 2].astype(np.float32)
    sel_vec = (a.astype(np.float32) / np.float32(N)).astype(np.float32)
    sel_node = (np.round(a).astype(np.int64) % N).astype(np.int32)
    return sel_vec, sel_node, lp, ent


LAST_RESULTS = None
# fp16 LDWEIGHTS is rejected by this walrus's ldw-opt pass
# (visitInstLdweights "not compatible" for any non-fp32 dtype), so the
# redundant-load elision stays off.  --max-sem-num shrinks the
# walrus-generated postamble semaphore-reset sweep (~115 ns per sem per
# engine lane), which otherwise burns ~8 us zeroing all 256 sems.
ENABLE_LDW_OPT = False
MAX_SEM_NUM = 176
_LDW_PATCHED = False


def _patch_ldw_opt():
    """Adjust walrus args (ldw-opt flip, postamble sem-sweep cap)."""
    global _LDW_PATCHED
    if _LDW_PATCHED:
        return
    import concourse.bass_utils as bu
    orig = bu.run_command

    def patched(argv, **kw):
        if ENABLE_LDW_OPT:
            argv = ["--enable-ldw-opt=true" if a == "--enable-ldw-opt=false"
                    else a for a in argv]
        if MAX_SEM_NUM and any("walrus_driver" in str(a) for a in argv[:1]):
            argv = list(argv) + [f"--max-sem-num={MAX_SEM_NUM}"]
        return orig(argv, **kw)

    bu.run_command = patched
    _LDW_PATCHED = True


def _run(inputs, trace=False):
    global LAST_RESULTS
    _patch_ldw_opt()
    from concourse.bass_utils import run_bass_kernel_spmd
    nc = _get_program()
    in_maps = _make_in_maps(inputs)
    res = run_bass_kernel_spmd(nc, in_maps, list(range(NCORES)), trace=trace)
    LAST_RESULTS = res
    return _postprocess([res.results[c]["res_out"] for c in range(NCORES)])


def kernel(**inputs):
    return _run(inputs, trace=False)


# revision 47
# speedup vs baseline: 1.0478x; 1.0478x over previous
"""Trainium2 Bass kernel for nn_Agent_57732950393167 (ragged_sequence).

Strategy (v2: fp16 data path)
-----------------------------
Data-parallel over batches: 32 batches / 8 cores = 4 batches ("groups" g)
per core, each with V=8 vehicles -> 32 vehicles/core.

The v1 kernel was PE-bound: fp32 moving operands stream at 2 cycles per
element on the PE, fp32 transposes and fp32-stationary matmuls run as
double (LOW+HIGH) passes, and LDWEIGHTS of fp32 stationaries cannot use
fast-weight-load.  v2 moves the whole heavy data path to fp16:

 * All large inputs ship as fp16 (halves HBM traffic to ~4.8 MB/core)
   and all large matmuls run with fp16 stationary+moving operands
   (1 cycle/element, 4x fast-weight-load for 128-col stationaries,
   single-pass transposes).  PSUM accumulation stays fp32.
 * Numerically validated offline: with fp16 rounding applied to every
   input AND every on-device cast point (query, qw, attention weights,
   AF, heads, final_Q, fw) the flat-softmax argmax of all 32 batches is
   unchanged and the min top-2 gap stays 6.5e-4 (fp64 ref: 4.1e-4).
   bf16 flips one batch -- fp16 is the floor.
 * nde = ndf @ W_ns ([T,N,384]) is never materialized (rank-8 folding
   into compat / heads / logits, as in v1).
 * Single sync-HWDGE DMA ring in consumption order; transfers merged
   into 11 issues (consts+weights fp16/f32, kt+ndftm group-pair halves,
   rhsha per pair, lt halves, mask).
 * Phase-A small ops moved from ACT to DVE (broadcast copies / STT) so
   the ACT queue reaches the first softmax exp immediately after C0.
 * Softmax runs unnormalized; 1/sum folded into the heads PSUM rescale.
 * log(mask) approximated by MASK_BIG*(mask-1), MASK_BIG=50 (fp16-exact).
"""

import numpy as np

B, N, D, H, V = 32, 1024, 128, 8, 8
KS = D // H            # 16
F_V = 4
F_ND = 8
TANH_CLIP = 10.0
MASK_BIG = 50.0
NCORES = 8
G = B // NCORES        # 4 groups (batches) per core
NPAIR = G // 2         # 2 batch-pairs per core

_PROGRAM_CACHE = {}

# fp16 const+weight pack A: everything phase A / C needs (cols)
CP16_REPL = 0          # [8,128]  eye(8) tiled 16x horizontally
CP16_HSELB = 128       # [128,128]
CP16_REPLBIG = 256     # [8,64]
CP16_WCSHI = 320       # [128,128]
CP16_WCSLO = 448       # [4,128]
CP16_VDFT = 576        # [4,32]
CP16_WNSKT = 608       # [128,8]
CP16A_W = 616

# fp16 const+weight pack B: late-use (T/sm phases)
CP16_IDENT = 0         # [128,128] identity
CP16_WOUT = 128        # [128,128]
CP16_WNSV = 256        # [8,128]
CP16_WNSLT = 384       # [128,8]
CP16B_W = 392

# f32 const pack A: phase A needs (cols)
CPF_FCT = 0            # [128,4]
CPF_BDSEL = 4          # [128,128]
CPFA_W = 132

# f32 const pack B: late-use
CPF_DIAG = 0           # [128,128]
CPF_IDENTPAD = 128     # [128,64]
CPF_HSEL = 192         # [128,64]
CPF_FWSEL = 256        # [128,16]
CPF_IDENT32 = 272      # [32,32]
CPF_C8K = 304          # [32,1]
CPFB_W = 305


def _build_cpack16a():
    cp = np.zeros((128, CP16A_W), dtype=np.float16)
    cp[0:F_ND, CP16_REPL:CP16_REPL + 128] = np.tile(
        np.eye(F_ND, dtype=np.float16), (1, 16))
    hb = np.zeros((128, 128), dtype=np.float16)
    for d in range(128):
        h = d // KS
        for g2 in range(2):
            hb[d, g2 * 64 + h * V:g2 * 64 + (h + 1) * V] = 1.0
    cp[:, CP16_HSELB:CP16_HSELB + 128] = hb
    rb = np.zeros((V, H * V), dtype=np.float16)
    for v in range(V):
        rb[v, v::V] = 1.0
    cp[0:V, CP16_REPLBIG:CP16_REPLBIG + 64] = rb
    return cp


def _build_cpack16b():
    cp = np.zeros((128, CP16B_W), dtype=np.float16)
    cp[:, CP16_IDENT:CP16_IDENT + 128] = np.eye(128, dtype=np.float16)
    return cp


def _build_cpackfa():
    return np.zeros((128, CPFA_W), dtype=np.float32)


def _build_cpackfb():
    cp = np.zeros((128, CPFB_W), dtype=np.float32)
    dm = np.zeros((128, 128), dtype=np.float32)
    for p in range(128):
        g2, hv = divmod(p, 64)
        a = g2 * 8 + (hv % 8)
        dm[p, a * 8:(a + 1) * 8] = 1.0
    cp[:, CPF_DIAG:CPF_DIAG + 128] = dm
    cp[:, CPF_IDENTPAD:CPF_IDENTPAD + 64] = np.tile(
        np.eye(64, dtype=np.float32), (2, 1))
    hs = np.zeros((128, 64), dtype=np.float32)
    for hk in range(128):
        h = hk // KS
        hs[hk, h * 8:(h + 1) * 8] = 1.0
    cp[:, CPF_HSEL:CPF_HSEL + 64] = hs
    fw = np.zeros((128, 16), dtype=np.float32)
    for p in range(128):
        fw[p, p // 8] = 1.0
    cp[:, CPF_FWSEL:CPF_FWSEL + 16] = fw
    cp[0:32, CPF_IDENT32:CPF_IDENT32 + 32] = np.eye(32, dtype=np.float32)
    cp[0:32, CPF_C8K] = 8192.0 - 1024.0 * (np.arange(32) % 8)
    return cp


def _build_bdsel():
    bd = np.zeros((128, 128), dtype=np.float32)
    for p in range(128):
        a = p // 8
        g2, v = divmod(a, 8)
        bd[p, g2 * 64 + v:g2 * 64 + 64:8] = 1.0
    return bd


# --------------------------------------------------------------------------
# Device program
# --------------------------------------------------------------------------

def _build_program():
    import contextlib

    import concourse.bacc as bacc
    import concourse.tile as tile
    import concourse.mybir as mybir

    dt = mybir.dt
    f32 = dt.float32
    f16 = dt.float16
    AF_EXP = mybir.ActivationFunctionType.Exp
    AF_LN = mybir.ActivationFunctionType.Ln
    AF_TANH = mybir.ActivationFunctionType.Tanh
    AF_COPY = mybir.ActivationFunctionType.Copy
    OP = mybir.AluOpType
    AX = mybir.AxisListType

    nc = bacc.Bacc("TRN2", target_bir_lowering=False, debug=False,
                   num_devices=NCORES)

    # ---- external inputs (per-core shards, host-prepped layouts) ----
    cp16a_in = nc.dram_tensor("cp16a_in", [128, CP16A_W], f16,
                              kind="ExternalInput")
    cp16b_in = nc.dram_tensor("cp16b_in", [128, CP16B_W], f16,
                              kind="ExternalInput")
    cpfa_in = nc.dram_tensor("cpfa_in", [128, CPFA_W], f32,
                             kind="ExternalInput")
    cpfb_in = nc.dram_tensor("cpfb_in", [128, CPFB_W], f32,
                             kind="ExternalInput")
    kt_in = nc.dram_tensor("kt_in", [128, G * N], f16, kind="ExternalInput")
    nd_in = nc.dram_tensor("nd_in", [72, G * N], f16, kind="ExternalInput")
    rh_in = nc.dram_tensor("rh_in", [128, NPAIR * 3 * N], f16,
                           kind="ExternalInput")
    lt_in = nc.dram_tensor("lt_in", [128, G * N], f16, kind="ExternalInput")
    mbs32_in = nc.dram_tensor("mbs32_in", [32, N], f32,
                              kind="ExternalInput")

    res_out = nc.dram_tensor("res_out", [G, 4], f32, kind="ExternalOutput")

    with tile.TileContext(nc) as tc:
        with contextlib.ExitStack() as ctx:
            sb = ctx.enter_context(tc.tile_pool(name="sb", bufs=1))
            scr = ctx.enter_context(tc.tile_pool(name="scr", bufs=4))
            acc = ctx.enter_context(
                tc.tile_pool(name="acc", bufs=2, space="PSUM"))
            tp = ctx.enter_context(
                tc.tile_pool(name="tp", bufs=3, space="PSUM"))
            flp = ctx.enter_context(
                tc.tile_pool(name="flp", bufs=1, space="PSUM"))

            def P(name, shape, dtype=f32):
                return sb.tile(shape, dtype, name=name, tag=name)

            def S(name, shape, dtype=f32):
                if shape[-1] >= 512:
                    return scr.tile(shape, dtype, name=name, tag="sbig",
                                    bufs=4)
                return scr.tile(shape, dtype, name=name, tag="ssml", bufs=8)

            # ================= persistent SBUF tiles =================
            cp16a = P("cp16a", [128, CP16A_W], f16)
            cp16b = P("cp16b", [128, CP16B_W], f16)
            cpfa = P("cpfa", [128, CPFA_W], f32)
            cpfb = P("cpfb", [128, CPFB_W], f32)
            kt = P("kt", [128, G * N], f16)
            nd = P("nd", [72, G * N], f16)
            rh = P("rh", [128, NPAIR * 3 * N], f16)
            lt = P("lt", [128, G * N], f16)
            mbs32 = P("mbs32", [32, N], f32)
            attnt = [P(f"attntp{p}", [128, N], f16) for p in range(NPAIR)]
            attnnt = [P(f"attnnt{p}", [128, 8 * 128], f16)
                      for p in range(NPAIR)]
            fctq = P("fctq", [128, G])
            fct8a = P("fct8a", [128, G * V], f16)
            queryt = P("queryt", [128, G * V], f16)  # 0.25-scaled query^T
            blockq = [P(f"blockq{p}", [128, 128], f16) for p in range(NPAIR)]
            bdq72 = [P(f"bdq72_{g}", [72, 64], f16) for g in range(G)]
            ha_sb = [P(f"hasb{p}", [128, 384]) for p in range(NPAIR)]
            afdt = [P(f"afdt{p}", [F_ND, 128], f16) for p in range(NPAIR)]
            hct = [P(f"hctp{p}", [128, 2 * V], f16) for p in range(NPAIR)]
            fqt = [P(f"fqt{p}", [128, 2 * V], f16) for p in range(NPAIR)]
            fq32g = [P(f"fq32g{g}", [128, 32], f16) for g in range(G)]
            bdfw32 = [P(f"bdfw32_{g}", [64, 32], f16) for g in range(G)]
            lgf32 = P("lgf32", [32, N])
            u32 = P("u32", [32, N])
            rinv_p = [P(f"rinvp{p}", [128, 1]) for p in range(NPAIR)]
            prime16 = P("prime16", [128, 256], f16)

            # ================= DMA issues =================
            # Everything rides ONE HWDGE ring (sync) in exact consumption
            # order: a single queue gets the full per-NC HBM bandwidth and
            # drains strictly FIFO, so arrival order == this issue order.
            nc.sync.dma_start(cp16a[:], cp16a_in.ap())
            nc.sync.dma_start(cpfa[:], cpfa_in.ap())
            nc.sync.dma_start(kt[:, 0:2 * N], kt_in.ap()[:, 0:2 * N])
            nc.sync.dma_start(nd[:, 0:2 * N], nd_in.ap()[:, 0:2 * N])
            nc.sync.dma_start(kt[:, 2 * N:4 * N], kt_in.ap()[:, 2 * N:4 * N])
            nc.sync.dma_start(nd[:, 2 * N:4 * N], nd_in.ap()[:, 2 * N:4 * N])
            nc.sync.dma_start(cp16b[:], cp16b_in.ap())
            nc.sync.dma_start(cpfb[:], cpfb_in.ap())
            nc.sync.dma_start(mbs32[:], mbs32_in.ap())
            nc.sync.dma_start(rh[:, 0:3 * N], rh_in.ap()[:, 0:3 * N])
            nc.sync.dma_start(rh[:, 3 * N:6 * N], rh_in.ap()[:, 3 * N:6 * N])
            nc.sync.dma_start(lt[:, 0:2 * N], lt_in.ap()[:, 0:2 * N])
            nc.sync.dma_start(lt[:, 2 * N:4 * N], lt_in.ap()[:, 2 * N:4 * N])

            # ================= small setup on DVE ==================
            res16 = P("res16", [1, 4 * G])
            nc.vector.memset(prime16[:], 0.0)
            nc.vector.memset(res16[:, 3:16:4], 0.0)
            for g in range(G):
                nc.vector.memset(fq32g[g][:], 0.0)
                nc.vector.memset(bdfw32[g][:], 0.0)

            # PE warm-up: back-to-back fp16 junk matmuls.  prime(n) chains
            # are interleaved INTO phase A's matmul sequence so the PE
            # stays near-continuously busy through A's cross-engine hops
            # (the HAM needs ~3.4us of sustained activity to un-throttle,
            # and 6 isolated primes measurably never warmed it, while 16
            # up-front primes blocked phase A for 4us).
            def prime(n):
                for i in range(n):
                    prime_ps = flp.tile([128, 256], f32,
                                        name=f"prime{next(_pc)}", tag="fl")
                    nc.tensor.matmul(prime_ps[:], prime16[:, 0:128],
                                     prime16[:], start=True, stop=True,
                                     skip_group_check=True)

            import itertools
            _pc = itertools.count()
            prime(4)

            # HAM keep-warm fillers: cheap fp16 junk matmuls into a
            # dedicated PSUM bank.  The tile scheduler floats dep-free
            # instructions, so fillers must READ a recently-produced tile
            # to stay pinned at their program point.
            fl_ps = flp.tile([128, 256], f32, name="fl_ps", tag="fl")

            def filler(n, early=False):
                for _ in range(n):
                    if early:
                        nc.tensor.matmul(fl_ps[:], prime16[:, 0:128],
                                         prime16[:],
                                         start=True, stop=True,
                                         skip_group_check=True)
                    else:
                        nc.tensor.matmul(fl_ps[:], kt[:, 0:128],
                                         kt[:, 0:256],
                                         start=True, stop=True,
                                         skip_group_check=True)

            def filler_on(st, mv, m, w):
                # junk matmul reading `st` (stationary [K, m]) and `mv`
                # (moving [K, w]) so the scheduler cannot hoist it ahead of
                # the ops that produce them
                nc.tensor.matmul(fl_ps[0:m, 0:w], st, mv,
                                 start=True, stop=True,
                                 skip_group_check=True)

            # const/weight slices
            repl = cp16a[0:F_ND, CP16_REPL:CP16_REPL + 128]
            hselb = cp16a[:, CP16_HSELB:CP16_HSELB + 128]
            replbig = cp16a[0:V, CP16_REPLBIG:CP16_REPLBIG + 64]
            wcs_hi = cp16a[:, CP16_WCSHI:CP16_WCSHI + 128]
            wcs_lo = cp16a[0:F_V, CP16_WCSLO:CP16_WCSLO + 128]
            vdft = cp16a[0:F_V, CP16_VDFT:CP16_VDFT + 32]
            wnskt = cp16a[:, CP16_WNSKT:CP16_WNSKT + F_ND]
            ident16 = cp16b[:, CP16_IDENT:CP16_IDENT + 128]
            wout = cp16b[:, CP16_WOUT:CP16_WOUT + 128]
            wnsv = cp16b[0:F_ND, CP16_WNSV:CP16_WNSV + 128]
            wnslt = cp16b[:, CP16_WNSLT:CP16_WNSLT + F_ND]
            fct = cpfa[:, CPF_FCT:CPF_FCT + 4]
            bdsel = cpfa[:, CPF_BDSEL:CPF_BDSEL + 128]
            diagmask = cpfb[:, CPF_DIAG:CPF_DIAG + 128]
            identpad = cpfb[:, CPF_IDENTPAD:CPF_IDENTPAD + 64]
            hsel = cpfb[:, CPF_HSEL:CPF_HSEL + 64]
            fwsel = cpfb[:, CPF_FWSEL:CPF_FWSEL + 16]
            ident32 = cpfb[0:32, CPF_IDENT32:CPF_IDENT32 + 32]
            c8k32 = cpfb[0:32, CPF_C8K:CPF_C8K + 1]

            # fctq = 0.25*fc^T (f32, used as ACT/STT bias)
            nc.vector.tensor_scalar_mul(fctq[:], fct, 0.25)

            # ================= phase A: query / qw smalls =================
            # fct8a[:, (g,v)] = fc[:, g]  (broadcast copy on DVE)
            nc.vector.tensor_copy(
                fct8a.rearrange("d (g v) -> d g v", g=G),
                fct.unsqueeze(2).broadcast_to([128, G, V]))
            qt_ps = tp.tile([128, G * V], f32, name="qt_ps", tag="tp")
            nc.tensor.matmul(qt_ps[:], wcs_hi, fct8a[:],
                             start=True, stop=False, skip_group_check=True)
            nc.tensor.matmul(qt_ps[:], wcs_lo, vdft,
                             start=False, stop=True, skip_group_check=True)
            prime(3)
            # queryt = 0.25*(cur + fc) = 0.25*qt + fctq   (DVE STT)
            nc.vector.scalar_tensor_tensor(
                queryt.rearrange("d (g v) -> d g v", g=G),
                qt_ps.rearrange("d (g v) -> d g v", g=G), 0.25,
                fctq.unsqueeze(2).broadcast_to([128, G, V]),
                op0=OP.mult, op1=OP.add)

            for p in range(NPAIR):
                # blockq[d, (g2,h,v)] = queryt[d, (g,v)] * (h == d//16)
                qview = (queryt[:, 2 * p * V:(2 * p + 2) * V]
                         .rearrange("d (g2 v) -> d g2 v", g2=2)
                         .unsqueeze(2).broadcast_to([128, 2, H, V]))
                nc.vector.tensor_tensor(
                    blockq[p].rearrange("d (g2 h v) -> d g2 h v", g2=2, h=H),
                    qview, hselb.rearrange("d (g2 h v) -> d g2 h v",
                                           g2=2, h=H),
                    OP.mult)
                # qw_all[f, (g2,h,v)] then replicate+mask into block-diag
                qw_ps = tp.tile([F_ND, 128], f32, name=f"qw_ps{p}", tag="tp")
                nc.tensor.matmul(qw_ps[:], wnskt, blockq[p][:],
                                 start=True, stop=True)
                qw_sbt = S(f"qw_sbt{p}", [F_ND, 128], f16)
                nc.vector.tensor_copy(qw_sbt[:], qw_ps[:])
                prime(2)
                qwr_ps = tp.tile([128, 128], f32, name=f"qwr_ps{p}", tag="tp")
                nc.tensor.matmul(qwr_ps[:], repl, qw_sbt[:],
                                 start=True, stop=True)
                prime(2)
                for g2 in range(2):
                    g = 2 * p + g2
                    gsl = slice(g2 * 64, (g2 + 1) * 64)
                    nc.vector.tensor_tensor(bdq72[g][0:64, :],
                                            qwr_ps[gsl, gsl],
                                            bdsel[gsl, gsl], OP.mult)
                    nc.vector.tensor_copy(bdq72[g][64:72, :], replbig)

            # ===== phases C/T/H/sm =====
            def phase_C(g):
                p, g2 = divmod(g, 2)
                gsl = slice(g2 * 64, (g2 + 1) * 64)
                compat = acc.tile([64, N], f32, name=f"compat{g}",
                                  tag="acc")
                # static first (kt arrives before nd in the stream);
                # dyn + mask in one 72-row contraction after
                for half in range(2):
                    sl = slice(g * N + half * 512, g * N + (half + 1) * 512)
                    osl = slice(half * 512, (half + 1) * 512)
                    nc.tensor.matmul(
                        compat[:, osl], blockq[p][:, gsl], kt[:, sl],
                        start=True, stop=False, skip_group_check=True)
                for half in range(2):
                    sl = slice(g * N + half * 512, g * N + (half + 1) * 512)
                    osl = slice(half * 512, (half + 1) * 512)
                    nc.tensor.matmul(
                        compat[:, osl], bdq72[g][:], nd[:, sl],
                        start=False, stop=True, skip_group_check=True)
                # unnormalized exp into the pair tile (|compat| < ~15)
                rsum = S(f"rsum{g}", [64, 1])
                nc.scalar.activation(attnt[p][gsl, :], compat[:],
                                     AF_EXP, accum_out=rsum[:])
                nc.vector.reciprocal(rinv_p[p][gsl, :], rsum[:])

            def phase_T(p):
                # attn^T -> attn_n: fp16 [128,128] transposes, paired copies
                for c2 in range(4):
                    at_ps = tp.tile([128, 256], f16,
                                    name=f"at_ps{p}_{c2}", tag="tp")
                    for j in range(2):
                        c = 2 * c2 + j
                        nc.tensor.matmul(
                            at_ps[:, j * 128:(j + 1) * 128],
                            attnt[p][:, c * 128:(c + 1) * 128],
                            ident16,
                            is_transpose=True,
                            start=True, stop=True,
                            skip_group_check=True)
                    dst = (attnnt[p]
                           .rearrange("q (c w) -> q c w", w=128)
                           [:, 2 * c2:2 * c2 + 2, :])
                    src_ap = at_ps.rearrange("q (j w) -> q j w", j=2)
                    if c2 % 2 == 0:
                        nc.scalar.activation(dst, src_ap, AF_COPY)
                    else:
                        nc.vector.tensor_copy(dst, src_ap)
                filler_on(attnnt[p][:, 0:16], attnnt[p][:, 0:256], 16, 256)

            def phase_H(p):
                # heads+AF over the 3 contiguous rh regions (2-dim free
                # AP); normalization via rinv in the PSUM->SBUF move
                ha_ps = tp.tile([128, 384], f32, name=f"ha_ps{p}", tag="tp")
                rhp = (rh[:, p * 3 * N:(p + 1) * 3 * N]
                       .rearrange("q (r w) -> q r w", r=3))
                for c in range(8):
                    nc.tensor.matmul(ha_ps[:],
                                     attnnt[p][:, c * 128:(c + 1) * 128],
                                     rhp[:, :, c * 128:(c + 1) * 128],
                                     start=(c == 0), stop=(c == 7))
                nc.vector.tensor_scalar_mul(ha_sb[p][:], ha_ps[:],
                                            rinv_p[p][:])
                filler_on(ha_sb[p][:, 0:16], ha_sb[p][:, 0:256], 16, 256)

            def phase_sm(p):
                # AF diag-extract -> AFd [128, F] -> AFd^T (fp16)
                aftmp = S(f"aftmp{p}", [128, 128])
                nc.vector.tensor_tensor(aftmp[:], ha_sb[p][:, 256:384],
                                        diagmask, OP.mult)
                afd = S(f"afd{p}", [128, F_ND], f16)
                with nc.allow_low_precision("fp16 AF validated offline"):
                    nc.vector.tensor_reduce(
                        afd[:], aftmp.rearrange("q (a f) -> q f a", f=F_ND),
                        AX.X, OP.add)
                filler_on(afd[:, 0:8], attnnt[p][:, 0:256], 8, 256)
                afd_ps = tp.tile([F_ND, 128], f16, name=f"afd_ps{p}",
                                 tag="tp")
                nc.tensor.matmul(afd_ps[:], afd[:], ident16,
                                 is_transpose=True, start=True, stop=True)
                nc.vector.tensor_copy(afdt[p][:], afd_ps[:])

                # heads -> hcT -> final_Q^T per group
                fqp = tp.tile([128, 2 * V], f32, name=f"fqp{p}", tag="tp")
                for g2 in range(2):
                    g = 2 * p + g2
                    hq_ps = tp.tile([128, 64], f32, name=f"hq_ps{g}",
                                    tag="tp")
                    nc.tensor.matmul(
                        hq_ps[:],
                        ha_sb[p][g2 * 64:(g2 + 1) * 64,
                                 g2 * 128:(g2 + 1) * 128],
                        identpad[g2 * 64:(g2 + 1) * 64, :],
                        is_transpose=True, start=True, stop=False,
                        skip_group_check=True)
                    nc.tensor.matmul(
                        hq_ps[:], wnsv,
                        afdt[p][:, g2 * 64:(g2 + 1) * 64],
                        start=False, stop=True, skip_group_check=True)
                    hqs = S(f"hqs{g}", [128, 64])
                    nc.vector.tensor_tensor(hqs[:], hq_ps[:], hsel,
                                            OP.mult)
                    with nc.allow_low_precision("fp16 heads validated offline"):
                        nc.vector.tensor_reduce(
                            hct[p][:, g2 * V:(g2 + 1) * V],
                            hqs.rearrange("q (hh v) -> q v hh", v=V),
                            AX.X, OP.add)
                    filler_on(hct[p][:, g2 * V:g2 * V + V],
                              attnnt[p][:, 0:256], V, 256)
                nc.tensor.matmul(fqp[:], wout, hct[p][:],
                                 start=True, stop=True)
                nc.vector.tensor_copy(fqt[p][:], fqp[:])
                for g2 in range(2):
                    g = 2 * p + g2
                    nc.vector.tensor_copy(
                        fq32g[g][:, g * 8:(g + 1) * 8],
                        fqp[:, g2 * V:(g2 + 1) * V])

                # block-diag fw
                fw_ps = tp.tile([F_ND, 2 * V], f32, name=f"fw_ps{p}",
                                tag="tp")
                nc.tensor.matmul(fw_ps[:], wnslt, fqt[p][:],
                                 start=True, stop=True)
                fw_sbt = S(f"fw_sbt{p}", [F_ND, 2 * V], f16)
                nc.vector.tensor_copy(fw_sbt[:], fw_ps[:])
                filler_on(fqt[p][:, 0:16], attnnt[p][:, 0:256], 16, 256)
                fwr_ps = tp.tile([128, 2 * V], f32, name=f"fwr_ps{p}",
                                 tag="tp")
                nc.tensor.matmul(fwr_ps[:], repl, fw_sbt[:],
                                 start=True, stop=True)
                for g2 in range(2):
                    g = 2 * p + g2
                    gsl = slice(g2 * 64, (g2 + 1) * 64)
                    vsl = slice(g2 * V, (g2 + 1) * V)
                    nc.vector.tensor_tensor(
                        bdfw32[g][:, g * 8:(g + 1) * 8],
                        fwr_ps[gsl, vsl], fwsel[gsl, vsl], OP.mult)

            # ======== phase E: batched logits for all 4 groups ===========
            # lg32[8g+v, n] = logits of (group g, vehicle v): group g's
            # stationaries are zero-padded to 32 cols at offset 8g so all
            # 16 matmuls accumulate into one [32, N] PSUM tile.  Emitted in
            # two clumps (pair 0's groups after sm0, pair 1's after sm1) so
            # the PE streams pair 0's logits while sm1's DVE chain runs.
            # Stationary-major: each stationary loads once and streams both
            # 512-halves back-to-back.  (lg32 is allocated after the compat
            # tiles so the acc pool's slot rotation stays forward-ordered.)
            def phase_E(p, lg32):
                for g in (2 * p, 2 * p + 1):
                    for half in range(2):
                        osl = slice(half * 512, (half + 1) * 512)
                        sl = slice(g * N + half * 512,
                                   g * N + (half + 1) * 512)
                        nc.tensor.matmul(
                            lg32[:, osl], bdfw32[g][:], nd[0:64, sl],
                            start=(g == 0), stop=False,
                            skip_group_check=True)
                for g in (2 * p, 2 * p + 1):
                    for half in range(2):
                        osl = slice(half * 512, (half + 1) * 512)
                        sl = slice(g * N + half * 512,
                                   g * N + (half + 1) * 512)
                        nc.tensor.matmul(
                            lg32[:, osl], fq32g[g][:], lt[:, sl],
                            start=False,
                            stop=(p == 1 and g == G - 1 and half == 1),
                            skip_group_check=True)

            phase_C(0)
            phase_C(1)
            # arrival-pinned fillers bridge the PE while C2 waits for its
            # WAR slot (exp0); they gate on the same DMA data C2 needs
            filler_on(kt[0:72, 2 * N:2 * N + 16],
                      kt[0:72, 2 * N:2 * N + 256], 16, 256)
            phase_C(2)
            filler_on(nd[0:72, 3 * N:3 * N + 16],
                      nd[0:72, 3 * N:3 * N + 256], 16, 256)
            phase_C(3)
            phase_T(0)
            phase_T(1)
            phase_H(0)
            phase_sm(0)
            lg32 = acc.tile([32, N], f32, name="lg32", tag="acc")
            phase_H(1)
            phase_E(0, lg32)
            phase_sm(1)
            phase_E(1, lg32)

            # lgf = 10*tanh(x/sqrt(D)) + mask, in halves so the DVE
            # mask-add of half 0 overlaps the ACT tanh of half 1
            for half in range(2):
                sl = slice(half * 512, (half + 1) * 512)
                nc.scalar.activation(u32[:, sl], lg32[:, sl], AF_TANH,
                                     scale=float(1.0 / np.sqrt(D)))
                nc.vector.scalar_tensor_tensor(
                    lgf32[:, sl], u32[:, sl], TANH_CLIP, mbs32[:, sl],
                    op0=OP.mult, op1=OP.add)

            # ============ epilogue: batched flat log-softmax/argmax ======
            # cde[32, 4] packs (max, rowsum, candidate) so a single PE
            # transpose brings all three into free-dim layout at once.
            cde = P("cde", [32, 4])
            rs32h = S("rs32h", [32, 2])
            expf = S("expfe", [32, N])
            for half in range(2):
                sl = slice(half * 512, (half + 1) * 512)
                nc.scalar.activation(expf[:, sl], lgf32[:, sl], AF_EXP,
                                     accum_out=rs32h[:, half:half + 1])
            nc.vector.tensor_tensor(cde[:, 1:2], rs32h[:, 0:1],
                                    rs32h[:, 1:2], OP.add)
            mx8 = S("mx8e", [32, 8])
            ix8 = S("ix8e", [32, 8], dt.uint32)
            nc.vector.max_with_indices(mx8[:], ix8[:], lgf32[:])
            nc.vector.tensor_copy(cde[:, 0:1], mx8[:, 0:1])
            idxf = S("idxfe", [32, 1])
            nc.vector.tensor_copy(idxf[:], ix8[:, 0:1])
            nc.vector.tensor_tensor(cde[:, 2:3], c8k32, idxf[:],
                                    OP.subtract)

            cde_ps = tp.tile([1, 96], f32, name="cde_ps", tag="tp")
            for j in range(3):
                nc.tensor.matmul(cde_ps[:, j * 32:(j + 1) * 32],
                                 cde[:, j:j + 1], ident32,
                                 is_transpose=True, start=True, stop=True,
                                 skip_group_check=True)
            cdet = S("cdete", [1, 96])
            nc.vector.tensor_copy(cdet[:], cde_ps[:])
            rmt = cdet[0:1, 0:32]
            rst = cdet[0:1, 32:64]
            cdt = cdet[0:1, 64:96]

            mt4 = S("mt4e", [1, G])
            nc.vector.tensor_reduce(mt4[:],
                                    rmt.rearrange("o (g v) -> o g v", g=G),
                                    AX.X, OP.max)
            s4 = S("s4e", [1, G])
            nc.vector.tensor_reduce(s4[:],
                                    rst.rearrange("o (g v) -> o g v", g=G),
                                    AX.X, OP.add)
            # em4 (Exp) BEFORE the Ln so the ACT table set switches exactly
            # once (a Ln->Exp order costs a third table load, measured)
            em4 = S("em4e", [1, G])
            nc.scalar.activation(em4[:], mt4[:], AF_EXP, scale=-1.0)
            # cand chain on DVE overlaps the ACT hops
            mtb = (mt4.unsqueeze(2).broadcast_to([1, G, V]))
            eq = S("eqe", [1, 32])
            nc.vector.tensor_tensor(
                eq.rearrange("o (g v) -> o g v", g=G),
                rmt.rearrange("o (g v) -> o g v", g=G), mtb, OP.is_equal)
            cs = S("cse", [1, 32])
            nc.vector.tensor_tensor(cs[:], eq[:], cdt, OP.mult)
            cm4 = S("cm4e", [1, G])
            nc.vector.tensor_reduce(cm4[:],
                                    cs.rearrange("o (g v) -> o g v", g=G),
                                    AX.X, OP.max)
            nc.vector.tensor_scalar(res16[:, 0:16:4], cm4[:], -1.0, 8192.0,
                                    OP.mult, OP.add)
            s4p = S("s4pe", [1, G])
            nc.vector.tensor_tensor(s4p[:], s4[:], em4[:], OP.mult)
            lns4 = S("lns4e", [1, G])
            nc.scalar.activation(lns4[:], s4p[:], AF_LN)
            prob4 = S("prob4e", [1, G])
            nc.vector.reciprocal(prob4[:], s4p[:])
            nc.vector.tensor_scalar_mul(res16[:, 1:16:4], lns4[:], -1.0)
            nc.vector.tensor_tensor(res16[:, 2:16:4], prob4[:], lns4[:],
                                    OP.mult)
            nc.sync.dma_start(
                res_out.ap().rearrange("a b -> (a b)").unsqueeze(0),
                res16[:])

    nc.compile()
    return nc


def _get_program():
    if "nc" not in _PROGRAM_CACHE:
        _PROGRAM_CACHE["nc"] = _build_program()
    return _PROGRAM_CACHE["nc"]


# --------------------------------------------------------------------------
# Host-side sharding / layout prep
# --------------------------------------------------------------------------

def _make_in_maps(inputs):
    gk = np.asarray(inputs["glimpse_K_static"], dtype=np.float32)
    gv = np.asarray(inputs["glimpse_V_static"], dtype=np.float32)
    lk = np.asarray(inputs["logit_K_static"], dtype=np.float32)
    ndf = np.asarray(inputs["node_dynamic_features"], dtype=np.float32)
    vdf = np.asarray(inputs["vehicle_dynamic_features"], dtype=np.float32)
    fc = np.asarray(inputs["fixed_context"], dtype=np.float32)
    msk = np.asarray(inputs["feasibility_mask"])
    w_cs = np.asarray(inputs["W_cs"], dtype=np.float32)
    w_ns = np.asarray(inputs["W_ns"], dtype=np.float32)
    w_out = np.asarray(inputs["W_out"], dtype=np.float32)

    cp16a_base = _build_cpack16a()
    cp16b_base = _build_cpack16b()
    cpfa_base = _build_cpackfa()
    cpfa_base[:, CPF_BDSEL:CPF_BDSEL + 128] = _build_bdsel()
    cpfb_base = _build_cpackfb()

    in_maps = []
    for c in range(NCORES):
        bs = slice(c * G, (c + 1) * G)
        kt = np.ascontiguousarray(
            gk[:, bs].transpose(1, 0, 3, 2).reshape(G, 128, N)
            .transpose(1, 0, 2).reshape(128, G * N)).astype(np.float16)
        lt = np.ascontiguousarray(
            lk[bs].transpose(0, 2, 1).transpose(1, 0, 2)
            .reshape(128, G * N)).astype(np.float16)
        vn = gv[:, bs].transpose(1, 2, 0, 3).reshape(G, N, 128)
        ndd = ndf[bs]                                   # [G, V, N, F]
        ndtm = np.zeros((G, 72, N), dtype=np.float16)
        ndtm[:, 0:64, :] = ndd.transpose(0, 1, 3, 2).reshape(G, 64, N)
        mbx = (msk[bs].astype(np.float32) - 1.0) * MASK_BIG   # [G, V, N]
        ndtm[:, 64:72, :] = mbx
        nd_in = np.ascontiguousarray(
            ndtm.transpose(1, 0, 2).reshape(72, G * N))
        mbs32 = np.ascontiguousarray(mbx.reshape(32, N)).astype(np.float32)
        ndfn = (ndd.reshape(NPAIR, 2, V, N, F_ND)
                .transpose(0, 3, 1, 2, 4).reshape(NPAIR, N, 128))
        rhsha = np.empty((NPAIR, 128, 24, 128), dtype=np.float16)
        for p in range(NPAIR):
            for g2 in range(2):
                rhsha[p, :, g2 * 8:(g2 + 1) * 8, :] = (
                    vn[2 * p + g2].reshape(8, 128, 128).transpose(1, 0, 2))
            rhsha[p, :, 16:24, :] = (
                ndfn[p].reshape(8, 128, 128).transpose(1, 0, 2))
        rh_in = np.ascontiguousarray(
            rhsha.reshape(NPAIR, 128, 3 * N)
            .transpose(1, 0, 2).reshape(128, NPAIR * 3 * N))

        cp16a = cp16a_base.copy()
        cp16a[:, CP16_WCSHI:CP16_WCSHI + 128] = w_cs[:D].astype(np.float16)
        cp16a[0:F_V, CP16_WCSLO:CP16_WCSLO + 128] = (
            w_cs[D:].astype(np.float16))
        cp16a[0:F_V, CP16_VDFT:CP16_VDFT + 32] = (
            vdf[bs].transpose(2, 0, 1).reshape(F_V, 32).astype(np.float16))
        cp16a[:, CP16_WNSKT:CP16_WNSKT + F_ND] = (
            w_ns[:, D:2 * D].T.astype(np.float16))
        cp16b = cp16b_base.copy()
        cp16b[:, CP16_WOUT:CP16_WOUT + 128] = w_out.astype(np.float16)
        cp16b[0:F_ND, CP16_WNSV:CP16_WNSV + 128] = (
            w_ns[:, 0:D].astype(np.float16))
        cp16b[:, CP16_WNSLT:CP16_WNSLT + F_ND] = (
            w_ns[:, 2 * D:3 * D].T.astype(np.float16))
        cpfa = cpfa_base.copy()
        cpfa[:, CPF_FCT:CPF_FCT + 4] = fc[bs].T

        in_maps.append({
            "cp16a_in": cp16a,
            "cp16b_in": cp16b,
            "cpfa_in": cpfa,
            "cpfb_in": cpfb_base,
            "kt_in": kt,
            "nd_in": nd_in,
            "rh_in": rh_in,
            "lt_in": lt,
            "mbs32_in": mbs32,
        })
    return in_maps


def _postprocess(res_list):
    out = np.concatenate(res_list, axis=0)              # [B, 4]
    a = out[:, 0]
    lp = out[:, 1].astype(np.float32)
    ent = out[:, 2].astype(np.float32)
    sel_vec = (a.astype(np.float32) / np.float32(N)).astype(np.float32)
    sel_node = (np.round(a).astype(np.int64) % N).astype(np.int32)
    return sel_vec, sel_node, lp, ent


LAST_RESULTS = None
# fp16 LDWEIGHTS is rejected by this walrus's ldw-opt pass
# (visitInstLdweights "not compatible" for any non-fp32 dtype), so the
# redundant-load elision stays off.  --max-sem-num shrinks the
# walrus-generated postamble semaphore-reset sweep (~115 ns per sem per
# engine lane), which otherwise burns ~8 us zeroing all 256 sems.
ENABLE_LDW_OPT = False
MAX_SEM_NUM = 176
_LDW_PATCHED = False


def _patch_ldw_opt():
    """Adjust walrus args (ldw-opt flip, postamble sem-sweep cap)."""
    global _LDW_PATCHED
    if _LDW_PATCHED:
        return
    import concourse.bass_utils as bu
    orig = bu.run_command

    def patched(argv, **kw):
        if ENABLE_LDW_OPT:
            argv = ["--enable-ldw-opt=true" if a == "--enable-ldw-opt=false"
                    else a for a in argv]
        if MAX_SEM_NUM and any("walrus_driver" in str(a) for a in argv[:1]):
            argv = list(argv) + [f"--max-sem-num={MAX_SEM_NUM}"]
        return orig(argv, **kw)

    bu.run_command = patched
    _LDW_PATCHED = True


def _run(inputs, trace=False):
    global LAST_RESULTS
    _patch_ldw_opt()
    from concourse.bass_utils import run_bass_kernel_spmd
    nc = _get_program()
    in_maps = _make_in_maps(inputs)
    res = run_bass_kernel_spmd(nc, in_maps, list(range(NCORES)), trace=trace)
    LAST_RESULTS = res
    return _postprocess([res.results[c]["res_out"] for c in range(NCORES)])


def kernel(**inputs):
    return _run(inputs, trace=False)
